# revision 1
# baseline (speedup 1.0000x reference)
"""Trainium2 Bass kernel for nn_Block_3599182594921 (gnn_message_passing).

Pure data parallel over batch B=32 across 8 NeuronCores (4 samples/core).
Stage 1 (prompt+transformer block) is batch-local; the irregular gathers
(idx / center_idx) address the whole batch, so each core's stage-1 output
rows are AllGathered into a replicated bf16 table in DRAM; each core then
gathers its rows with indirect DMA and runs group attention / pooling /
propagate / adapter locally.

Activations are feature-major ([C-chunk, token]) so GEMMs run without
per-layer transposes; fp32r for the main GEMMs, bf16 for attention internals,
the MLP and the gather table, fp32 for residual streams.
"""

import contextlib

import numpy as np
import ml_dtypes

import concourse.bacc as bacc
import concourse.bass as bass
import concourse.tile as tile
from concourse import mybir
from concourse.bass_utils import run_bass_kernel_spmd
from concourse.masks import make_identity

f32 = mybir.dt.float32
f32r = mybir.dt.float32r
bf16 = mybir.dt.bfloat16
i32 = mybir.dt.int32
AX = mybir.AxisListType
OP = mybir.AluOpType
ACTF = mybir.ActivationFunctionType

N_CORES = 8
B, G, S, T, C, GK, H, HD = 32, 512, 128, 16, 384, 32, 6, 64
BPC = B // N_CORES          # 4 samples per core
N = T + G                   # 528 tokens per sample in stage 1
ROWS = BPC * N              # 2112 table rows per core
TBL = B * N                 # 16896 table rows total
R = 16                      # adapter bottleneck
H4 = 4 * C                  # 1536
NEG = -100000.0
SCALE = HD ** -0.5

TQ = 264                    # stage-1 moving-dim chunk (2 per sample)
TKS = [128, 128, 128, 128, 16]


def _bcast(nc, ppT, ones_row, row, n):
    """Broadcast [1, n] f32r row to [128, n] psum via K=1 matmul."""
    pb = ppT.tile([128, n], f32, tag="t", name="bc")
    nc.tensor.matmul(out=pb[:], lhsT=ones_row[:1, :], rhs=row[:1, :n],
                     start=True, stop=True)
    return pb


def _ln_fm(nc, sb1, sb3, ppT, xT, ntok, ones_col_f, ones_row, g_col, b_col,
           out, name, ones_sq=None):
    """LayerNorm over C of feature-major xT [128, 3, ntok] -> out [128,3,ntok].

    Stats via PE ones-matmuls (partition reduction); everything per 264-chunk.
    """
    nq = (ntok + TQ - 1) // TQ
    for q in range(nq):
        q0 = q * TQ
        qn = min(TQ, ntok - q0)
        ps = ppT.tile([1, TQ], f32, tag="t", name="lnp1")
        for ch in range(3):
            nc.tensor.matmul(out=ps[:, :qn], lhsT=ones_col_f[:, :1],
                             rhs=xT[:, ch, q0:q0 + qn],
                             start=(ch == 0), stop=(ch == 2))
        mean = sb3.tile([1, TQ], f32, tag="ln_mean", name="lnmean", bufs=1)
        nc.scalar.mul(out=mean[:, :qn], in_=ps[:, :qn], mul=1.0 / C)
        ps2 = ppT.tile([1, TQ], f32, tag="t", name="lnp2")
        for ch in range(3):
            sq = sb3.tile([128, TQ], f32, tag="ln_sq", name="lnsq", bufs=1)
            nc.vector.tensor_tensor(out=sq[:, :qn], in0=xT[:, ch, q0:q0 + qn],
                                    in1=xT[:, ch, q0:q0 + qn], op=OP.mult)
            nc.tensor.matmul(out=ps2[:, :qn],
                             lhsT=(ones_sq if ones_sq is not None
                                   else ones_col_f)[:, :1],
                             rhs=sq[:, :qn], start=(ch == 0), stop=(ch == 2))
        var = sb3.tile([1, TQ], f32, tag="ln_var", name="lnvar", bufs=1)
        # var = E[x^2] - mean^2 ; then rstd = 1/sqrt(var+eps)
        nc.vector.scalar_tensor_tensor(out=var[:, :qn], in0=mean[:, :qn],
                                       scalar=-1.0, in1=mean[:, :qn],
                                       op0=OP.mult, op1=OP.mult)
        nc.vector.scalar_tensor_tensor(out=var[:, :qn], in0=ps2[:, :qn],
                                       scalar=1.0 / C, in1=var[:, :qn],
                                       op0=OP.mult, op1=OP.add)
        nc.vector.tensor_scalar(out=var[:, :qn], in0=var[:, :qn], scalar1=1e-5,
                                scalar2=None, op0=OP.add)
        nc.scalar.activation(out=var[:, :qn], in_=var[:, :qn], func=ACTF.Sqrt)
        nc.vector.reciprocal(out=var[:, :qn], in_=var[:, :qn])   # rstd
        r_rstd = sb3.tile([1, TQ], f32r, tag="ln_rr", name="lnrr", bufs=1)
        nc.vector.tensor_copy(out=r_rstd[:, :qn], in_=var[:, :qn])
        r_mr = sb3.tile([1, TQ], f32r, tag="ln_mr", name="lnmr", bufs=1)
        nc.vector.tensor_tensor(out=r_mr[:, :qn], in0=mean[:, :qn],
                                in1=var[:, :qn], op=OP.mult)
        b_rstd = _bcast(nc, ppT, ones_row, r_rstd[:, :qn], qn)
        b_mr = _bcast(nc, ppT, ones_row, r_mr[:, :qn], qn)
        for ch in range(3):
            t = sb3.tile([128, TQ], f32, tag="ln_t", name="lnt", bufs=1)
            nc.vector.tensor_tensor(out=t[:, :qn], in0=xT[:, ch, q0:q0 + qn],
                                    in1=b_rstd[:, :qn], op=OP.mult)
            nc.vector.tensor_tensor(out=t[:, :qn], in0=t[:, :qn],
                                    in1=b_mr[:, :qn], op=OP.subtract)
            nc.vector.tensor_scalar(out=out[:, ch, q0:q0 + qn], in0=t[:, :qn],
                                    scalar1=g_col[:, ch:ch + 1],
                                    scalar2=b_col[:, ch:ch + 1],
                                    op0=OP.mult, op1=OP.add)


def build(debug_outputs=()):
    nc = bacc.Bacc("TRN2", target_bir_lowering=False, debug=False,
                   num_devices=N_CORES)
    dd = {}

    def din(name, shape, dtype=f32):
        dd[name] = nc.dram_tensor(name, shape, dtype, kind="ExternalInput")
        return dd[name]

    din("xinT", [BPC, 3, 128, N])
    din("maskT", [BPC, N, N], bf16)
    din("idx", [BPC * S * GK, 1], i32)
    din("cidx", [BPC * S, 1], i32)
    din("c1T", [BPC, 3, G])
    din("n1sq", [BPC, 1, G])
    din("c2T", [BPC, 3, S])
    din("n2sq", [BPC, S, 1])
    din("wqkvT", [3, 128, 3 * C])
    din("wprojT", [3, 128, C])
    din("bproj", [3, 128])
    din("wfc1T", [3, 128, H4], bf16)
    din("bfc1", [12, 128])
    din("wfc2T", [12, 128, C], bf16)
    din("bfc2", [3, 128])
    din("waddT", [3, 128, R])
    din("adb", [R, 1])
    din("waduT", [R, C])
    din("adub", [3, 128])
    din("wa1dT", [3, 128, R])
    din("a1db", [R, 1])
    din("wa1uT", [R, C])
    din("a1ub", [3, 128])
    din("wa1qkvT", [3, 128, 3 * C], bf16)
    din("wa1projT", [3, 128, C], bf16)
    din("ba1proj", [3, 128])
    din("gn1", [3, 128]), din("bn1c", [3, 128])
    din("gn2", [3, 128]), din("bn2c", [3, 128])
    din("gn3r", [3, 128]), din("bn3r", [3, 128])
    din("bnscale", [3, 128]), din("bnbias", [3, 128])
    din("gate", [1, 1])
    din("blockmask", [128, 512])
    y = nc.dram_tensor("y", [BPC, G, C], f32, kind="ExternalOutput")
    dbg = {}
    for dn, shape in debug_outputs:
        dbg[dn] = nc.dram_tensor(dn, shape, f32, kind="ExternalOutput")

    with tile.TileContext(nc) as tc:
        ctx = contextlib.ExitStack()
        with ctx:
            dram = ctx.enter_context(tc.tile_pool(name="dram", bufs=1,
                                                  space="DRAM"))
            wp = ctx.enter_context(tc.tile_pool(name="wp", bufs=1))

            in_b = dram.tile([ROWS, C], bf16)
            table = dram.tile([TBL, C], bf16, addr_space="Shared")

            # ---- load weights (staging pool freed after this block) ----
            with tc.tile_pool(name="wst", bufs=1) as wst:
                def load_w(name, chunks, width, dtype):
                    src = dd[name]
                    if dtype == bf16:
                        w = wp.tile([128, chunks, width], bf16,
                                    name=f"w_{name}")
                        nc.sync.dma_start(out=w[:], in_=src[:, :, :].rearrange(
                            "a p x -> p a x"))
                        return w
                    stg = wst.tile([128, chunks, width], f32, tag="wstage",
                                   name=f"stg_{name}")
                    nc.sync.dma_start(out=stg[:], in_=src[:, :, :].rearrange(
                        "a p x -> p a x"))
                    w = wp.tile([128, chunks, width], dtype, name=f"w_{name}")
                    nc.vector.tensor_copy(out=w[:], in_=stg[:])
                    return w

                wqkv = load_w("wqkvT", 3, 3 * C, f32r)
                wproj = load_w("wprojT", 3, C, f32r)
                wfc1 = load_w("wfc1T", 3, H4, bf16)
                wfc2 = load_w("wfc2T", 12, C, bf16)
                wadd = load_w("waddT", 3, R, f32r)
                wa1d = load_w("wa1dT", 3, R, f32r)
                wa1qkv = load_w("wa1qkvT", 3, 3 * C, bf16)
                wa1proj = load_w("wa1projT", 3, C, bf16)

                def load_small(name, p, w, dtype=f32):
                    t = wp.tile([p, w], dtype, name=f"sm_{name}")
                    nc.sync.dma_start(out=t[:], in_=dd[name][:, :])
                    return t

                wadu_f = wst.tile([R, C], f32, tag="usmall", name="wadu_f")
                nc.sync.dma_start(out=wadu_f[:], in_=dd["waduT"][:, :])
                wadu = wp.tile([R, C], f32r)
                nc.vector.tensor_copy(out=wadu[:], in_=wadu_f[:])
                wa1u_f = wst.tile([R, C], f32, tag="usmall2", name="wa1u_f")
                nc.sync.dma_start(out=wa1u_f[:], in_=dd["wa1uT"][:, :])
                wa1u = wp.tile([R, C], f32r)
                nc.vector.tensor_copy(out=wa1u[:], in_=wa1u_f[:])

                cols = {}
                for cn, p in [("bproj", 3), ("bfc1", 12), ("bfc2", 3),
                              ("adub", 3), ("a1ub", 3), ("ba1proj", 3),
                              ("gn1", 3), ("bn1c", 3), ("gn2", 3), ("bn2c", 3),
                              ("bnscale", 3), ("bnbias", 3)]:
                    t = wp.tile([128, p], f32, name=f"col_{cn}")
                    nc.sync.dma_start(out=t[:],
                                      in_=dd[cn][:, :].rearrange("a p -> p a"))
                    cols[cn] = t
                adb_c = load_small("adb", R, 1)
                a1db_c = load_small("a1db", R, 1)
                gate_c = wp.tile([128, 1], f32)
                nc.sync.dma_start(out=gate_c[:],
                                  in_=dd["gate"][:, :].to_broadcast([128, 1]))
                for cn in ("gn3r", "bn3r"):
                    t = wp.tile([128, 3], f32, name=f"col_{cn}")
                    nc.sync.dma_start(out=t[:],
                                      in_=dd[cn][:, :].rearrange("a p -> p a"))
                    cols[cn] = t
                bmask = wp.tile([128, 512], f32)
                nc.sync.dma_start(out=bmask[:], in_=dd["blockmask"][:, :])

            sb1 = ctx.enter_context(tc.tile_pool(name="sb1", bufs=1))
            sb2 = ctx.enter_context(tc.tile_pool(name="sb2", bufs=2))
            sb3 = ctx.enter_context(tc.tile_pool(name="sb3", bufs=3))
            ppT = ctx.enter_context(tc.tile_pool(name="ppT", bufs=4,
                                                 space="PSUM"))
            ppP = ctx.enter_context(tc.tile_pool(name="ppP", bufs=4,
                                                 space="PSUM"))

            ones_col_f = wp.tile([128, 1], f32)
            nc.vector.memset(ones_col_f[:], 1.0)
            ones_col = wp.tile([128, 1], f32r)
            nc.vector.tensor_copy(out=ones_col[:], in_=ones_col_f[:])
            ones_col_b = wp.tile([128, 1], bf16)
            nc.vector.memset(ones_col_b[:], 1.0)
            ones_row_f = wp.tile([1, 128], f32)
            nc.vector.memset(ones_row_f[:], 1.0)
            ones_row = wp.tile([1, 128], f32r)
            nc.vector.tensor_copy(out=ones_row[:], in_=ones_row_f[:])
            ones_row_b = wp.tile([1, 64], bf16)
            nc.vector.memset(ones_row_b[:], 1.0)
            ident = wp.tile([128, 128], f32)
            make_identity(nc, ident)
            ident_b = wp.tile([128, 128], bf16)
            nc.vector.tensor_copy(out=ident_b[:], in_=ident[:])

            x2d = dram.tile([BPC, 128, 3, N], f32)

            # head h lives in partition half h%2; slot h//2 (q) / 3+h//2 (k)
            def hslice(qk_tile, h, is_k, t0, tn):
                po = (h % 2) * 64
                slot = (3 if is_k else 0) + h // 2
                return qk_tile[po:po + 64, slot, t0:t0 + tn]

            # ================= STAGE 1 =================
            for s in range(BPC):
                x0T = sb1.tile([128, 3, N], f32, tag="x0T")
                nc.sync.dma_start(out=x0T[:],
                                  in_=dd["xinT"][s, :, :, :].rearrange(
                                      "a p x -> p a x"))
                ln1T = sb1.tile([128, 3, N], f32r, tag="ln1T")
                _ln_fm(nc, sb1, sb3, ppT, x0T, N, ones_col_f, ones_row,
                       cols["gn1"], cols["bn1c"], ln1T, "ln1")
                qkT = sb1.tile([128, 6, N], bf16, tag="qkT")
                for h in range(H):
                    for is_k in (0, 1):
                        po = (h % 2) * 64
                        f0 = (is_k * C) + h * 64
                        for tq in range(2):
                            t0 = tq * TQ
                            ps = ppT.tile([64, TQ], f32, tag="t", name="qkp")
                            for ch in range(3):
                                nc.tensor.matmul(
                                    out=ps[:],
                                    lhsT=wqkv[:, ch, f0:f0 + 64],
                                    rhs=ln1T[:, ch, t0:t0 + TQ],
                                    start=(ch == 0), stop=(ch == 2))
                            if po == 0:
                                nc.vector.tensor_copy(
                                    out=hslice(qkT, h, is_k, t0, TQ),
                                    in_=ps[:])
                            else:
                                qtmp = sb3.tile([64, TQ], bf16, tag="qtmp",
                                                name="qtmp", bufs=2)
                                nc.vector.tensor_copy(out=qtmp[:], in_=ps[:])
                                nc.sync.dma_start(
                                    out=hslice(qkT, h, is_k, t0, TQ),
                                    in_=qtmp[:])
                v1 = sb1.tile([128, 5, H, HD], bf16, tag="v1")
                for j, tk in enumerate(TKS):
                    t0 = j * 128
                    ps = ppT.tile([128, C], f32, tag="t", name="vp")
                    for ch in range(3):
                        nc.tensor.matmul(
                            out=ps[:tk, :], lhsT=ln1T[:, ch, t0:t0 + tk],
                            rhs=wqkv[:, ch, 2 * C:3 * C],
                            start=(ch == 0), stop=(ch == 2))
                    nc.vector.tensor_copy(
                        out=v1[:tk, j, :, :],
                        in_=ps[:tk, :].rearrange("p (h d) -> p h d", h=H))
                mt = sb1.tile([128, 5, N], bf16, tag="maskt")
                for j, tk in enumerate(TKS):
                    nc.sync.dma_start(out=mt[:tk, j, :],
                                      in_=dd["maskT"][s, j * 128:j * 128 + tk, :])
                attn_nT = sb1.tile([128, 3, N], f32r, tag="ln1T",
                                   name="attn_nT")
                for h in range(H):
                    po = (h % 2) * 64
                    den = [ppP.tile([1, TQ], f32, tag="p", name=f"den{t}")
                           for t in range(2)]
                    att = [ppP.tile([64, TQ], f32, tag="p", name=f"att{t}")
                           for t in range(2)]
                    for j, tk in enumerate(TKS):
                        t0 = j * 128
                        for tq in range(2):
                            q0 = tq * TQ
                            st = ppT.tile([128, TQ], f32, tag="t", name="st")
                            nc.tensor.matmul(out=st[:tk, :],
                                             lhsT=hslice(qkT, h, 1, t0, tk),
                                             rhs=hslice(qkT, h, 0, q0, TQ),
                                             start=True, stop=True)
                            lg = sb3.tile([128, TQ], f32, tag="lg", name="lg", bufs=1)
                            nc.vector.scalar_tensor_tensor(
                                out=lg[:tk, :], in0=st[:tk, :], scalar=SCALE,
                                in1=mt[:tk, j, q0:q0 + TQ],
                                op0=OP.mult, op1=OP.add)
                            ex = sb3.tile([128, TQ], bf16, tag="ex", name="ex", bufs=2)
                            nc.scalar.activation(out=ex[:tk, :], in_=lg[:tk, :],
                                                 func=ACTF.Exp)
                            nc.tensor.matmul(out=den[tq][:],
                                             lhsT=ones_col_b[:tk, :],
                                             rhs=ex[:tk, :], start=(j == 0),
                                             stop=(j == 4))
                            nc.tensor.matmul(out=att[tq][:],
                                             lhsT=v1[:tk, j, h, :],
                                             rhs=ex[:tk, :], start=(j == 0),
                                             stop=(j == 4))
                    for tq in range(2):
                        q0 = tq * TQ
                        rr = sb3.tile([1, TQ], f32r, tag="rr")
                        with nc.allow_low_precision(reason="softmax recip"):
                            nc.vector.reciprocal(out=rr[:], in_=den[tq][:])
                        bc = ppT.tile([64, TQ], f32, tag="t", name="bcq")
                        nc.tensor.matmul(out=bc[:],
                                         lhsT=ones_row[:1, :64],
                                         rhs=rr[:1, :], start=True, stop=True)
                        bcs = sb3.tile([64, TQ], f32, tag="bcs", name="bcs",
                                       bufs=1)
                        nc.scalar.copy(out=bcs[:], in_=bc[:])
                        if po == 0:
                            nc.vector.tensor_tensor(
                                out=attn_nT[0:64, h // 2, q0:q0 + TQ],
                                in0=att[tq][:], in1=bcs[:], op=OP.mult)
                        else:
                            ntmp = sb3.tile([64, TQ], f32r, tag="ntmp",
                                            name="ntmp", bufs=2)
                            nc.vector.tensor_tensor(out=ntmp[:], in0=att[tq][:],
                                                    in1=bcs[:], op=OP.mult)
                            nc.sync.dma_start(
                                out=attn_nT[64:128, h // 2, q0:q0 + TQ],
                                in_=ntmp[:])
                x1T = x0T
                for f in range(3):
                    for tq in range(2):
                        q0 = tq * TQ
                        ps = ppT.tile([128, TQ], f32, tag="t", name="pjp")
                        for ch in range(3):
                            nc.tensor.matmul(
                                out=ps[:],
                                lhsT=wproj[:, ch, f * 128:(f + 1) * 128],
                                rhs=attn_nT[:, ch, q0:q0 + TQ],
                                start=(ch == 0), stop=(ch == 2))
                        nc.vector.scalar_tensor_tensor(
                            out=x1T[:, f, q0:q0 + TQ], in0=ps[:],
                            scalar=cols["bproj"][:, f:f + 1],
                            in1=x0T[:, f, q0:q0 + TQ], op0=OP.add, op1=OP.add)
                ln2T = sb1.tile([128, 3, N], bf16, tag="ln2T")
                _ln_fm(nc, sb1, sb3, ppT, x1T, N, ones_col_f, ones_row,
                       cols["gn2"], cols["bn2c"], ln2T, "ln2")
                xfnT = sb1.tile([128, 3, N], f32r, tag="xfnT")
                for tq in range(2):
                    q0 = tq * TQ
                    h1T = sb1.tile([128, 12, TQ], bf16, tag="h1T")
                    for fh in range(12):
                        ps = ppT.tile([128, TQ], f32, tag="t", name="f1p")
                        for ch in range(3):
                            nc.tensor.matmul(
                                out=ps[:],
                                lhsT=wfc1[:, ch, fh * 128:(fh + 1) * 128],
                                rhs=ln2T[:, ch, q0:q0 + TQ],
                                start=(ch == 0), stop=(ch == 2))
                        nc.scalar.activation(out=h1T[:, fh, :], in_=ps[:],
                                             func=ACTF.Gelu,
                                             bias=cols["bfc1"][:, fh:fh + 1],
                                             scale=1.0)
                    for f in range(3):
                        ps = ppT.tile([128, TQ], f32, tag="t", name="f2p")
                        for ch in range(12):
                            nc.tensor.matmul(
                                out=ps[:],
                                lhsT=wfc2[:, ch, f * 128:(f + 1) * 128],
                                rhs=h1T[:, ch, :],
                                start=(ch == 0), stop=(ch == 11))
                        nc.scalar.activation(out=xfnT[:, f, q0:q0 + TQ],
                                             in_=ps[:], func=ACTF.Identity,
                                             bias=cols["bfc2"][:, f:f + 1],
                                             scale=1.0)
                x2T = sb1.tile([128, 3, N], f32, tag="x2T")
                for tq in range(2):
                    q0 = tq * TQ
                    psd = ppT.tile([R, TQ], f32, tag="t", name="adp")
                    for ch in range(3):
                        nc.tensor.matmul(out=psd[:], lhsT=wadd[:, ch, :],
                                         rhs=xfnT[:, ch, q0:q0 + TQ],
                                         start=(ch == 0), stop=(ch == 2))
                    d0 = sb3.tile([R, TQ], f32r, tag="d0", name="d0", bufs=1)
                    nc.scalar.activation(out=d0[:], in_=psd[:], func=ACTF.Gelu,
                                         bias=adb_c[:, :1], scale=1.0)
                    for f in range(3):
                        psu = ppT.tile([128, TQ], f32, tag="t", name="aup")
                        nc.tensor.matmul(out=psu[:],
                                         lhsT=wadu[:, f * 128:(f + 1) * 128],
                                         rhs=d0[:], start=True, stop=True)
                        tt = sb3.tile([128, TQ], f32, tag="adt", name="tt", bufs=1)
                        nc.vector.scalar_tensor_tensor(
                            out=tt[:], in0=psu[:],
                            scalar=cols["adub"][:, f:f + 1],
                            in1=xfnT[:, f, q0:q0 + TQ], op0=OP.add, op1=OP.add)
                        nc.vector.scalar_tensor_tensor(
                            out=x2T[:, f, q0:q0 + TQ], in0=tt[:],
                            scalar=gate_c[:, :1], in1=x1T[:, f, q0:q0 + TQ],
                            op0=OP.mult, op1=OP.add)
                nc.sync.dma_start(out=x2d[s, :, :, :], in_=x2T[:])
                x2b = sb1.tile([128, 3, 640], bf16, tag="x2b")
                nc.vector.memset(x2b[:, :, 512:], 0.0)
                nc.vector.tensor_copy(out=x2b[:, :, :N], in_=x2T[:])
                for j, tk in enumerate(TKS):
                    t0 = j * 128
                    tm = sb3.tile([128, 3, 128], bf16, tag="tm", name="tm",
                                  bufs=3)
                    for ch in range(3):
                        nc.sync.dma_start_transpose(
                            out=tm[:, ch, :], in_=x2b[:, ch, t0:t0 + 128])
                    nc.sync.dma_start(
                        out=in_b[s * N + t0:s * N + t0 + tk, :],
                        in_=tm[:tk, :, :].rearrange("t a p -> t (a p)"))

            if "d_x2" in dbg:
                for s in range(BPC):
                    nc.sync.dma_start(out=dbg["d_x2"][s, :, :, :],
                                      in_=x2d[s, :, :, :].rearrange("p a x -> a p x"))

            # ================= ALLGATHER =================
            nc.gpsimd.collective_compute(
                "AllGather", OP.bypass,
                replica_groups=[list(range(N_CORES))],
                ins=[in_b.opt()], outs=[table.opt()])

            # ================= STAGE 2+3 =================
            for s in range(BPC):
                cix = sb1.tile([S, 1], i32, tag="cix")
                nc.sync.dma_start(out=cix[:],
                                  in_=dd["cidx"][s * S:(s + 1) * S, :])
                xc = sb1.tile([S, C], bf16, tag="xc")
                nc.gpsimd.indirect_dma_start(
                    out=xc[:], out_offset=None, in_=table[:],
                    in_offset=bass.IndirectOffsetOnAxis(ap=cix[:, :1], axis=0))
                xcT = sb1.tile([128, 3, S], bf16, tag="xcT")
                for ch in range(3):
                    nc.sync.dma_start_transpose(
                        out=xcT[:, ch, :], in_=xc[:, ch * 128:(ch + 1) * 128])
                vis_xT = sb1.tile([128, 3, S], f32, tag="vis_xT")

                for cki in range(8):
                    base = (s * 8 + cki) * 512
                    gT = sb2.tile([128, 3, 512], bf16, tag="gT")
                    ln3T = sb1.tile([128, 3, 512], bf16, tag="ln3T")
                    gsubs = []
                    for sub in range(4):
                        ixt = sb3.tile([128, 1], i32, tag="ixt")
                        nc.sync.dma_start(
                            out=ixt[:],
                            in_=dd["idx"][base + sub * 128:
                                          base + sub * 128 + 128, :])
                        g = sb3.tile([128, C], bf16, tag="gsub", name="gsub",
                                     bufs=4)
                        nc.gpsimd.indirect_dma_start(
                            out=g[:], out_offset=None, in_=table[:],
                            in_offset=bass.IndirectOffsetOnAxis(
                                ap=ixt[:, :1], axis=0))
                        gsubs.append(g)
                        if s == 0 and cki == 0 and sub == 0 and "d_g0" in dbg:
                            og = sb3.tile([128, C], f32, tag="l3", name="dbgg",
                                          bufs=2)
                            nc.vector.tensor_copy(out=og[:], in_=g[:])
                            nc.sync.dma_start(out=dbg["d_g0"][:, :], in_=og[:])
                        for ch in range(3):
                            pt = ppT.tile([128, 128], bf16, tag="t", name="gtp")
                            nc.tensor.transpose(
                                out=pt[:], in_=g[:, ch * 128:(ch + 1) * 128],
                                identity=ident_b[:])
                            nc.vector.tensor_copy(
                                out=gT[:, ch, sub * 128:sub * 128 + 128],
                                in_=pt[:])
                    # LN3 feature-major on gT (stats via PE ones-matmuls)
                    _ln_fm(nc, sb1, sb3, ppT, gT, 512, ones_col_b, ones_row,
                           cols["gn3r"], cols["bn3r"], ln3T, "ln3",
                           ones_sq=ones_col_f)
                    qk2 = sb1.tile([128, 6, 512], bf16, tag="qk2")
                    for h in range(H):
                        for is_k in (0, 1):
                            po = (h % 2) * 64
                            f0 = (is_k * C) + h * 64
                            ps = ppT.tile([64, 512], f32, tag="t", name="qk2p")
                            for ch in range(3):
                                nc.tensor.matmul(
                                    out=ps[:],
                                    lhsT=wa1qkv[:, ch, f0:f0 + 64],
                                    rhs=ln3T[:, ch, :],
                                    start=(ch == 0), stop=(ch == 2))
                            if po == 0:
                                nc.scalar.copy(
                                    out=hslice(qk2, h, is_k, 0, 512), in_=ps[:])
                            else:
                                qtmp = sb3.tile([64, 512], bf16, tag="qtmp2",
                                                name="qtmp2", bufs=1)
                                nc.scalar.copy(out=qtmp[:], in_=ps[:])
                                nc.sync.dma_start(
                                    out=hslice(qk2, h, is_k, 0, 512),
                                    in_=qtmp[:])
                    v2 = sb1.tile([128, 4, H, HD], bf16, tag="v2")
                    for sub in range(4):
                        ps = ppT.tile([128, C], f32, tag="t", name="v2p")
                        for ch in range(3):
                            nc.tensor.matmul(
                                out=ps[:],
                                lhsT=ln3T[:, ch, sub * 128:sub * 128 + 128],
                                rhs=wa1qkv[:, ch, 2 * C:3 * C],
                                start=(ch == 0), stop=(ch == 2))
                        nc.vector.tensor_copy(
                            out=v2[:, sub, :, :],
                            in_=ps[:].rearrange("p (h d) -> p h d", h=H))
                    # block-diagonal attention, batched over the 4 sub-blocks
                    at2 = sb1.tile([128, 3, 512], bf16, tag="at2")
                    for h in range(H):
                        po = (h % 2) * 64
                        stb = ppT.tile([128, 512], f32, tag="t", name="st2")
                        for sub in range(4):
                            o0 = sub * 128
                            nc.tensor.matmul(out=stb[:, o0:o0 + 128],
                                             lhsT=hslice(qk2, h, 1, o0, 128),
                                             rhs=hslice(qk2, h, 0, o0, 128),
                                             start=True, stop=True)
                        lg = sb3.tile([128, 512], f32, tag="lg2", name="lg2",
                                      bufs=1)
                        nc.vector.scalar_tensor_tensor(
                            out=lg[:], in0=stb[:], scalar=SCALE,
                            in1=bmask[:], op0=OP.mult, op1=OP.add)
                        ex = sb3.tile([128, 512], bf16, tag="ex2", name="ex2",
                                      bufs=2)
                        nc.scalar.activation(out=ex[:], in_=lg[:], func=ACTF.Exp)
                        den = ppP.tile([1, 512], f32, tag="p", name="den2")
                        nc.tensor.matmul(out=den[:], lhsT=ones_col_b[:, :1],
                                         rhs=ex[:], start=True, stop=True)
                        att = ppP.tile([64, 512], f32, tag="p", name="att2")
                        for sub in range(4):
                            o0 = sub * 128
                            nc.tensor.matmul(out=att[:, o0:o0 + 128],
                                             lhsT=v2[:, sub, h, :],
                                             rhs=ex[:, o0:o0 + 128],
                                             start=True, stop=True)
                        rr = sb3.tile([1, 512], bf16, tag="rr2", name="rr2",
                                      bufs=1)
                        with nc.allow_low_precision(reason="softmax recip"):
                            nc.vector.reciprocal(out=rr[:], in_=den[:])
                        bcs = sb3.tile([64, 512], bf16, tag="bcs2",
                                       name="bcs2", bufs=1)
                        nc.gpsimd.partition_broadcast(bcs[:], rr[:1, :],
                                                      channels=64)
                        if po == 0:
                            nc.vector.tensor_tensor(
                                out=at2[0:64, h // 2, :],
                                in0=att[:], in1=bcs[:], op=OP.mult)
                        else:
                            ntmp = sb3.tile([64, 512], bf16, tag="ntmp2",
                                            name="ntmp2", bufs=1)
                            nc.vector.tensor_tensor(out=ntmp[:], in0=att[:],
                                                    in1=bcs[:], op=OP.mult)
                            nc.sync.dma_start(out=at2[64:128, h // 2, :],
                                              in_=ntmp[:])
                    if s == 0 and cki == 0 and "d_at2" in dbg:
                        for ch in range(3):
                            ot = sb3.tile([128, 512], f32, tag="xnn",
                                          name="dbg_at", bufs=1)
                            nc.vector.tensor_copy(out=ot[:], in_=at2[:, ch, :])
                            nc.sync.dma_start(out=dbg["d_at2"][ch, :, :], in_=ot[:])
                    lc_all = sb3.tile([128, 3, 16], f32, tag="lcall",
                                      name="lcall", bufs=2)
                    for f in range(3):
                        ps = ppT.tile([128, 512], f32, tag="t", name="pj2")
                        for ch in range(3):
                            nc.tensor.matmul(
                                out=ps[:],
                                lhsT=wa1proj[:, ch, f * 128:(f + 1) * 128],
                                rhs=at2[:, ch, :], start=(ch == 0),
                                stop=(ch == 2))
                        xnn = sb3.tile([128, 512], f32, tag="xnn", name="xnn",
                                       bufs=1)
                        nc.vector.scalar_tensor_tensor(
                            out=xnn[:], in0=ps[:],
                            scalar=cols["ba1proj"][:, f:f + 1],
                            in1=gT[:, f, :], op0=OP.add, op1=OP.add)
                        pm = sb3.tile([128, 16], f32, tag="pm")
                        nc.vector.tensor_reduce(
                            out=pm[:],
                            in_=xnn[:].rearrange("p (g k) -> p g k", k=GK),
                            axis=AX.X, op=OP.max)
                        pa = sb3.tile([128, 16], f32, tag="pa")
                        nc.vector.tensor_reduce(
                            out=pa[:],
                            in_=xnn[:].rearrange("p (g k) -> p g k", k=GK),
                            axis=AX.X, op=OP.add)
                        lc = sb3.tile([128, 16], f32, tag="lc")
                        nc.vector.scalar_tensor_tensor(
                            out=lc[:], in0=pa[:], scalar=1.0 / GK, in1=pm[:],
                            op0=OP.mult, op1=OP.add)
                        nc.scalar.activation(out=lc_all[:, f, :], in_=lc[:],
                                             func=ACTF.Gelu,
                                             bias=cols["bnbias"][:, f:f + 1],
                                             scale=cols["bnscale"][:, f:f + 1])
                    c0 = cki * 16
                    nc.vector.scalar_tensor_tensor(
                        out=vis_xT[:, :, c0:c0 + 16],
                        in0=xcT[:, :, c0:c0 + 16], scalar=0.4, in1=lc_all[:],
                        op0=OP.mult, op1=OP.add)

                # ---- stage 3 ----
                vis_x = sb1.tile([128, 3, S], f32r, tag="vis_x")
                for ch in range(3):
                    pt = ppT.tile([128, 128], f32, tag="t", name="vtp")
                    nc.tensor.transpose(out=pt[:], in_=vis_xT[:, ch, :],
                                        identity=ident[:])
                    nc.vector.tensor_copy(out=vis_x[:, ch, :], in_=pt[:])
                if "d_vis" in dbg:
                    for ch in range(3):
                        ot = sb3.tile([128, S], f32, tag="ob", name="dbv", bufs=2)
                        nc.vector.tensor_copy(out=ot[:], in_=vis_x[:, ch, :])
                        nc.sync.dma_start(out=dbg["d_vis"][s, ch, :, :],
                                          in_=ot[:])
                c2t = sb1.tile([3, S], f32, tag="c2t")
                nc.sync.dma_start(out=c2t[:], in_=dd["c2T"][s, :, :])
                c1t = sb1.tile([3, G], f32, tag="c1t")
                nc.sync.dma_start(out=c1t[:], in_=dd["c1T"][s, :, :])
                n1r = sb1.tile([1, G], f32, tag="n1r")
                nc.sync.dma_start(out=n1r[:], in_=dd["n1sq"][s, :, :])
                n1rr = sb1.tile([1, G], f32r, tag="n1rr")
                nc.vector.tensor_copy(out=n1rr[:], in_=n1r[:])
                n2c = sb1.tile([S, 1], f32, tag="n2c")
                nc.sync.dma_start(out=n2c[:], in_=dd["n2sq"][s, :, :])
                psd = ppP.tile([S, G], f32, tag="p", name="dps")
                nc.tensor.matmul(out=psd[:], lhsT=c2t[:3, :], rhs=c1t[:3, :],
                                 start=True, stop=True)
                nb = _bcast(nc, ppT, ones_row, n1rr, G)
                nbs = sb1.tile([S, G], f32, tag="nbs")
                nc.scalar.copy(out=nbs[:], in_=nb[:])
                dT = sb1.tile([S, G], f32, tag="dT")
                nc.vector.scalar_tensor_tensor(out=dT[:], in0=psd[:],
                                               scalar=-2.0, in1=nbs[:],
                                               op0=OP.mult, op1=OP.add)
                nc.vector.tensor_scalar(out=dT[:], in0=dT[:],
                                        scalar1=n2c[:, :1], scalar2=None,
                                        op0=OP.add)
                rT = sb1.tile([S, G], f32r, tag="rT")
                with nc.allow_low_precision(reason="propagate recip"):
                    nc.vector.reciprocal(out=rT[:], in_=dT[:])
                pss = ppP.tile([1, G], f32, tag="p", name="rsum")
                nc.tensor.matmul(out=pss[:], lhsT=ones_col[:, :1], rhs=rT[:],
                                 start=True, stop=True)
                rs = sb1.tile([1, G], f32r, tag="rs")
                with nc.allow_low_precision(reason="propagate recip"):
                    nc.vector.reciprocal(out=rs[:], in_=pss[:])
                rb = _bcast(nc, ppT, ones_row, rs, G)
                wT = sb1.tile([S, G], f32r, tag="wTn")
                nc.vector.tensor_tensor(out=wT[:], in0=rT[:], in1=rb[:],
                                        op=OP.mult)
                xgT = sb1.tile([128, 3, G], f32, tag="xgT")
                nc.sync.dma_start(out=xgT[:], in_=x2d[s, :, :, T:])
                nxT = sb1.tile([128, 3, G], f32r, tag="nxT")
                for ch in range(3):
                    psi = ppT.tile([128, G], f32, tag="t", name="ips")
                    nc.tensor.matmul(out=psi[:], lhsT=vis_x[:, ch, :],
                                     rhs=wT[:], start=True, stop=True)
                    nc.vector.scalar_tensor_tensor(
                        out=nxT[:, ch, :], in0=psi[:], scalar=0.4,
                        in1=xgT[:, ch, :], op0=OP.mult, op1=OP.add)
                psa = ppT.tile([R, G], f32, tag="t", name="a1dp")
                for ch in range(3):
                    nc.tensor.matmul(out=psa[:], lhsT=wa1d[:, ch, :],
                                     rhs=nxT[:, ch, :], start=(ch == 0),
                                     stop=(ch == 2))
                d1 = sb1.tile([R, G], f32r, tag="d1")
                nc.scalar.activation(out=d1[:], in_=psa[:], func=ACTF.Gelu,
                                     bias=a1db_c[:, :1], scale=1.0)
                for ch in range(3):
                    psu = ppT.tile([128, G], f32, tag="t", name="a1up")
                    nc.tensor.matmul(out=psu[:],
                                     lhsT=wa1u[:, ch * 128:(ch + 1) * 128],
                                     rhs=d1[:], start=True, stop=True)
                    oT = sb3.tile([128, G], f32, tag="oT", name="oT", bufs=2)
                    nc.vector.scalar_tensor_tensor(
                        out=oT[:], in0=psu[:],
                        scalar=cols["a1ub"][:, ch:ch + 1],
                        in1=nxT[:, ch, :], op0=OP.add, op1=OP.add)
                    for j in range(4):
                        pt = ppT.tile([128, 128], f32, tag="t", name="otp")
                        nc.tensor.transpose(out=pt[:],
                                            in_=oT[:, j * 128:(j + 1) * 128],
                                            identity=ident[:])
                        ob = sb3.tile([128, 128], f32, tag="ob", name="ob", bufs=2)
                        nc.vector.tensor_copy(out=ob[:], in_=pt[:])
                        nc.sync.dma_start(
                            out=y[s, j * 128:(j + 1) * 128,
                                  ch * 128:(ch + 1) * 128],
                            in_=ob[:])

    nc.compile()
    return nc


_CACHE = {}


def _get_nc():
    if "nc" not in _CACHE:
        _CACHE["nc"] = build()
    return _CACHE["nc"]


def prep_inputs(inputs):
    """Host-side prep: shard over batch, transpose to device layouts."""
    xx = {k: np.asarray(v) for k, v in inputs.items()}
    bf = ml_dtypes.bfloat16
    shared = {}
    shared["wqkvT"] = np.ascontiguousarray(xx["qkv_w"].T).reshape(3, 128, 3 * C)
    shared["wprojT"] = np.ascontiguousarray(xx["proj_w"].T).reshape(3, 128, C)
    shared["bproj"] = xx["proj_b"].reshape(3, 128)
    shared["wfc1T"] = np.ascontiguousarray(xx["fc1_w"].T).reshape(
        3, 128, H4).astype(bf)
    shared["bfc1"] = xx["fc1_b"].reshape(12, 128)
    shared["wfc2T"] = np.ascontiguousarray(xx["fc2_w"].T).reshape(
        12, 128, C).astype(bf)
    shared["bfc2"] = xx["fc2_b"].reshape(3, 128)
    shared["waddT"] = np.ascontiguousarray(xx["ad_dw"].T).reshape(3, 128, R)
    shared["adb"] = xx["ad_db"].reshape(R, 1)
    shared["waduT"] = np.ascontiguousarray(xx["ad_uw"].T).reshape(R, C)
    shared["adub"] = xx["ad_ub"].reshape(3, 128)
    shared["wa1dT"] = np.ascontiguousarray(xx["ad1_dw"].T).reshape(3, 128, R)
    shared["a1db"] = xx["ad1_db"].reshape(R, 1)
    shared["wa1uT"] = np.ascontiguousarray(xx["ad1_uw"].T).reshape(R, C)
    shared["a1ub"] = xx["ad1_ub"].reshape(3, 128)
    shared["wa1qkvT"] = np.ascontiguousarray(xx["a1_qkv_w"].T).reshape(
        3, 128, 3 * C).astype(bf)
    shared["wa1projT"] = np.ascontiguousarray(xx["a1_proj_w"].T).reshape(
        3, 128, C).astype(bf)
    shared["ba1proj"] = xx["a1_proj_b"].reshape(3, 128)
    shared["gn1"] = xx["n1_g"].reshape(3, 128)
    shared["bn1c"] = xx["n1_b"].reshape(3, 128)
    shared["gn2"] = xx["n2_g"].reshape(3, 128)
    shared["bn2c"] = xx["n2_b"].reshape(3, 128)
    shared["gn3r"] = xx["n3_g"].reshape(3, 128)
    shared["bn3r"] = xx["n3_b"].reshape(3, 128)
    shared["bnscale"] = (xx["bn_g"] / np.sqrt(np.float32(1.0 + 1e-5))
                         ).reshape(3, 128)
    shared["bnbias"] = xx["bn_b"].reshape(3, 128)
    shared["gate"] = xx["ad_gate"].reshape(1, 1)
    bm = np.full((128, 128), NEG, np.float32)
    for g in range(4):
        bm[g * 32:(g + 1) * 32, g * 32:(g + 1) * 32] = 0.0
    shared["blockmask"] = np.tile(bm, (1, 4))
    shared = {k: np.ascontiguousarray(v) for k, v in shared.items()}

    prompt = xx["prompt_embeddings"]
    idx = xx["idx"].reshape(B, S, GK)
    cidx = xx["center_idx"].reshape(B, S)
    maps = []
    for c in range(N_CORES):
        sl = slice(c * BPC, (c + 1) * BPC)
        xin = np.concatenate(
            [np.broadcast_to(prompt[None], (BPC, T, C)), xx["x"][sl]], axis=1)
        m = {}
        m["xinT"] = np.ascontiguousarray(xin.transpose(0, 2, 1)).reshape(
            BPC, 3, 128, N)
        mp = np.zeros((BPC, N, N), np.float32)
        mp[:, :G, :G] = NEG * xx["mask"][sl]
        m["maskT"] = np.ascontiguousarray(mp.transpose(0, 2, 1)).astype(bf)
        m["idx"] = np.ascontiguousarray(idx[sl].reshape(-1, 1))
        m["cidx"] = np.ascontiguousarray(cidx[sl].reshape(-1, 1))
        c1 = xx["center1"][sl]
        c2 = xx["center2"][sl]
        m["c1T"] = np.ascontiguousarray(c1.transpose(0, 2, 1))
        m["n1sq"] = np.ascontiguousarray((c1 ** 2).sum(-1)[:, None, :])
        m["c2T"] = np.ascontiguousarray(c2.transpose(0, 2, 1))
        m["n2sq"] = np.ascontiguousarray((c2 ** 2).sum(-1)[:, :, None] + 1e-8)
        m.update(shared)
        maps.append({k: np.ascontiguousarray(v) for k, v in m.items()})
    return maps


def run(maps, nc=None, debug_outputs=()):
    if nc is None:
        nc = _get_nc()
    res = run_bass_kernel_spmd(nc, maps, core_ids=list(range(N_CORES)))
    return res.results


def kernel(**inputs):
    maps = prep_inputs(inputs)
    results = run(maps)
    out = np.concatenate([r["y"] for r in results], axis=0)
    return out.astype(np.float32)



# revision 3
# speedup vs baseline: 34.8303x; 34.8303x over previous
"""Trainium2 Bass kernel for nn_Block_3599182594921 (gnn_message_passing).

Pure data parallel over batch B=32 across 8 NeuronCores (4 samples/core).
Stage 1 (prompt+transformer block) is batch-local; the irregular gathers
(idx / center_idx) address the whole batch, so each core's stage-1 output
rows are AllGathered into a replicated bf16 table in DRAM; each core then
gathers its rows with indirect DMA and runs group attention / pooling /
propagate / adapter locally.

Activations are feature-major ([C-chunk, token]) so GEMMs run without
per-layer transposes; fp32r for the main GEMMs, bf16 for attention internals,
the MLP and the gather table, fp32 for residual streams.
"""

import contextlib

import numpy as np
import ml_dtypes

import concourse.bacc as bacc
import concourse.bass as bass
import concourse.tile as tile
from concourse import mybir
from concourse.bass_utils import run_bass_kernel_spmd
from concourse.masks import make_identity

f32 = mybir.dt.float32
f32r = mybir.dt.float32r
bf16 = mybir.dt.bfloat16
i32 = mybir.dt.int32
AX = mybir.AxisListType
OP = mybir.AluOpType
ACTF = mybir.ActivationFunctionType

N_CORES = 8
B, G, S, T, C, GK, H, HD = 32, 512, 128, 16, 384, 32, 6, 64
BPC = B // N_CORES          # 4 samples per core
N = T + G                   # 528 tokens per sample in stage 1
ROWS = BPC * N              # 2112 table rows per core
TBL = B * N                 # 16896 table rows total
R = 16                      # adapter bottleneck
H4 = 4 * C                  # 1536
NEG = -100000.0
SCALE = HD ** -0.5

TQ = 264                    # stage-1 moving-dim chunk (2 per sample)
TKS = [128, 128, 128, 128, 16]


def _bcast(nc, ppT, ones_row, row, n):
    """Broadcast [1, n] f32r row to [128, n] psum via K=1 matmul."""
    pb = ppT.tile([128, n], f32, tag="t", name="bc")
    nc.tensor.matmul(out=pb[:], lhsT=ones_row[:1, :], rhs=row[:1, :n],
                     start=True, stop=True)
    return pb


def _ln_fm(nc, sb1, sb3, ppT, xT, ntok, ones_col_f, ones_row, g_col, b_col,
           out, name, ones_sq=None):
    """LayerNorm over C of feature-major xT [128, 3, ntok] -> out [128,3,ntok].

    Stats via PE ones-matmuls (partition reduction); everything per 264-chunk.
    """
    nq = (ntok + TQ - 1) // TQ
    for q in range(nq):
        q0 = q * TQ
        qn = min(TQ, ntok - q0)
        ps = ppT.tile([1, TQ], f32, tag="t", name="lnp1")
        for ch in range(3):
            nc.tensor.matmul(out=ps[:, :qn], lhsT=ones_col_f[:, :1],
                             rhs=xT[:, ch, q0:q0 + qn],
                             start=(ch == 0), stop=(ch == 2))
        mean = sb3.tile([1, TQ], f32, tag="ln_mean", name="lnmean", bufs=1)
        nc.scalar.mul(out=mean[:, :qn], in_=ps[:, :qn], mul=1.0 / C)
        ps2 = ppT.tile([1, TQ], f32, tag="t", name="lnp2")
        for ch in range(3):
            sq = sb3.tile([128, TQ], f32, tag="ln_sq", name="lnsq", bufs=1)
            nc.vector.tensor_tensor(out=sq[:, :qn], in0=xT[:, ch, q0:q0 + qn],
                                    in1=xT[:, ch, q0:q0 + qn], op=OP.mult)
            nc.tensor.matmul(out=ps2[:, :qn],
                             lhsT=(ones_sq if ones_sq is not None
                                   else ones_col_f)[:, :1],
                             rhs=sq[:, :qn], start=(ch == 0), stop=(ch == 2))
        var = sb3.tile([1, TQ], f32, tag="ln_var", name="lnvar", bufs=1)
        # var = E[x^2] - mean^2 ; then rstd = 1/sqrt(var+eps)
        nc.vector.scalar_tensor_tensor(out=var[:, :qn], in0=mean[:, :qn],
                                       scalar=-1.0, in1=mean[:, :qn],
                                       op0=OP.mult, op1=OP.mult)
        nc.vector.scalar_tensor_tensor(out=var[:, :qn], in0=ps2[:, :qn],
                                       scalar=1.0 / C, in1=var[:, :qn],
                                       op0=OP.mult, op1=OP.add)
        nc.vector.tensor_scalar(out=var[:, :qn], in0=var[:, :qn], scalar1=1e-5,
                                scalar2=None, op0=OP.add)
        nc.scalar.activation(out=var[:, :qn], in_=var[:, :qn], func=ACTF.Sqrt)
        nc.vector.reciprocal(out=var[:, :qn], in_=var[:, :qn])   # rstd
        r_rstd = sb3.tile([1, TQ], f32r, tag="ln_rr", name="lnrr", bufs=1)
        nc.vector.tensor_copy(out=r_rstd[:, :qn], in_=var[:, :qn])
        r_mr = sb3.tile([1, TQ], f32r, tag="ln_mr", name="lnmr", bufs=1)
        nc.vector.tensor_tensor(out=r_mr[:, :qn], in0=mean[:, :qn],
                                in1=var[:, :qn], op=OP.mult)
        b_rstd = _bcast(nc, ppT, ones_row, r_rstd[:, :qn], qn)
        b_mr = _bcast(nc, ppT, ones_row, r_mr[:, :qn], qn)
        for ch in range(3):
            t = sb3.tile([128, TQ], f32, tag="ln_t", name="lnt", bufs=1)
            nc.vector.tensor_tensor(out=t[:, :qn], in0=xT[:, ch, q0:q0 + qn],
                                    in1=b_rstd[:, :qn], op=OP.mult)
            nc.vector.tensor_tensor(out=t[:, :qn], in0=t[:, :qn],
                                    in1=b_mr[:, :qn], op=OP.subtract)
            nc.vector.tensor_scalar(out=out[:, ch, q0:q0 + qn], in0=t[:, :qn],
                                    scalar1=g_col[:, ch:ch + 1],
                                    scalar2=b_col[:, ch:ch + 1],
                                    op0=OP.mult, op1=OP.add)


def build(debug_outputs=(), profile_no_collective=False):
    nc = bacc.Bacc("TRN2", target_bir_lowering=False, debug=False,
                   num_devices=N_CORES)
    dd = {}

    def din(name, shape, dtype=f32):
        dd[name] = nc.dram_tensor(name, shape, dtype, kind="ExternalInput")
        return dd[name]

    din("xinT", [BPC, 3, 128, N])
    din("maskT", [BPC, N, N], bf16)
    din("idx", [BPC * S * GK, 1], i32)
    din("cidx", [BPC * S, 1], i32)
    din("c1T", [BPC, 3, G])
    din("n1sq", [BPC, 1, G])
    din("c2T", [BPC, 3, S])
    din("n2sq", [BPC, S, 1])
    din("wqkvT", [3, 128, 3 * C])
    din("wprojT", [3, 128, C])
    din("bproj", [3, 128])
    din("wfc1T", [3, 128, H4], bf16)
    din("bfc1", [12, 128])
    din("wfc2T", [12, 128, C], bf16)
    din("bfc2", [3, 128])
    din("waddT", [3, 128, R])
    din("adb", [R, 1])
    din("waduT", [R, C])
    din("adub", [3, 128])
    din("wa1dT", [3, 128, R])
    din("a1db", [R, 1])
    din("wa1uT", [R, C])
    din("a1ub", [3, 128])
    din("wa1qkvT", [3, 128, 3 * C], bf16)
    din("wa1projT", [3, 128, C], bf16)
    din("ba1proj", [3, 128])
    din("gn1", [3, 128]), din("bn1c", [3, 128])
    din("gn2", [3, 128]), din("bn2c", [3, 128])
    din("gn3r", [3, 128]), din("bn3r", [3, 128])
    din("bnscale", [3, 128]), din("bnbias", [3, 128])
    din("gate", [1, 1])
    din("blockmask", [128, 512])
    y = nc.dram_tensor("y", [BPC, G, C], f32, kind="ExternalOutput")
    dbg = {}
    for dn, shape in debug_outputs:
        dbg[dn] = nc.dram_tensor(dn, shape, f32, kind="ExternalOutput")

    with tile.TileContext(nc) as tc:
        ctx = contextlib.ExitStack()
        with ctx:
            dram = ctx.enter_context(tc.tile_pool(name="dram", bufs=1,
                                                  space="DRAM"))
            wp = ctx.enter_context(tc.tile_pool(name="wp", bufs=1))

            in_b = dram.tile([ROWS, C], bf16)
            table = dram.tile([TBL, C], bf16, addr_space="Shared")

            # ---- load weights (staging pool freed after this block) ----
            with tc.tile_pool(name="wst", bufs=1) as wst:
                def load_w(name, chunks, width, dtype):
                    src = dd[name]
                    if dtype == bf16:
                        w = wp.tile([128, chunks, width], bf16,
                                    name=f"w_{name}")
                        nc.sync.dma_start(out=w[:], in_=src[:, :, :].rearrange(
                            "a p x -> p a x"))
                        return w
                    stg = wst.tile([128, chunks, width], f32, tag="wstage",
                                   name=f"stg_{name}")
                    nc.sync.dma_start(out=stg[:], in_=src[:, :, :].rearrange(
                        "a p x -> p a x"))
                    w = wp.tile([128, chunks, width], dtype, name=f"w_{name}")
                    nc.vector.tensor_copy(out=w[:], in_=stg[:])
                    return w

                wqkv = load_w("wqkvT", 3, 3 * C, f32r)
                wproj = load_w("wprojT", 3, C, f32r)
                wfc1 = load_w("wfc1T", 3, H4, bf16)
                wfc2 = load_w("wfc2T", 12, C, bf16)
                wadd = load_w("waddT", 3, R, f32r)
                wa1d = load_w("wa1dT", 3, R, f32r)
                wa1qkv = load_w("wa1qkvT", 3, 3 * C, bf16)
                wa1proj = load_w("wa1projT", 3, C, bf16)

                def load_small(name, p, w, dtype=f32):
                    t = wp.tile([p, w], dtype, name=f"sm_{name}")
                    nc.sync.dma_start(out=t[:], in_=dd[name][:, :])
                    return t

                wadu_f = wst.tile([R, C], f32, tag="usmall", name="wadu_f")
                nc.sync.dma_start(out=wadu_f[:], in_=dd["waduT"][:, :])
                wadu = wp.tile([R, C], f32r)
                nc.vector.tensor_copy(out=wadu[:], in_=wadu_f[:])
                wa1u_f = wst.tile([R, C], f32, tag="usmall2", name="wa1u_f")
                nc.sync.dma_start(out=wa1u_f[:], in_=dd["wa1uT"][:, :])
                wa1u = wp.tile([R, C], f32r)
                nc.vector.tensor_copy(out=wa1u[:], in_=wa1u_f[:])

                cols = {}
                for cn, p in [("bproj", 3), ("bfc1", 12), ("bfc2", 3),
                              ("adub", 3), ("a1ub", 3), ("ba1proj", 3),
                              ("gn1", 3), ("bn1c", 3), ("gn2", 3), ("bn2c", 3),
                              ("bnscale", 3), ("bnbias", 3)]:
                    t = wp.tile([128, p], f32, name=f"col_{cn}")
                    nc.sync.dma_start(out=t[:],
                                      in_=dd[cn][:, :].rearrange("a p -> p a"))
                    cols[cn] = t
                adb_c = load_small("adb", R, 1)
                a1db_c = load_small("a1db", R, 1)
                gate_c = wp.tile([128, 1], f32)
                nc.sync.dma_start(out=gate_c[:],
                                  in_=dd["gate"][:, :].to_broadcast([128, 1]))
                for cn in ("gn3r", "bn3r"):
                    t = wp.tile([128, 3], f32, name=f"col_{cn}")
                    nc.sync.dma_start(out=t[:],
                                      in_=dd[cn][:, :].rearrange("a p -> p a"))
                    cols[cn] = t
                bmask = wp.tile([128, 512], f32)
                nc.sync.dma_start(out=bmask[:], in_=dd["blockmask"][:, :])

            sb1 = ctx.enter_context(tc.tile_pool(name="sb1", bufs=1))
            sb2 = ctx.enter_context(tc.tile_pool(name="sb2", bufs=2))
            sb3 = ctx.enter_context(tc.tile_pool(name="sb3", bufs=3))
            ppT = ctx.enter_context(tc.tile_pool(name="ppT", bufs=4,
                                                 space="PSUM"))
            ppP = ctx.enter_context(tc.tile_pool(name="ppP", bufs=4,
                                                 space="PSUM"))

            ones_col_f = wp.tile([128, 1], f32)
            nc.vector.memset(ones_col_f[:], 1.0)
            ones_col = wp.tile([128, 1], f32r)
            nc.vector.tensor_copy(out=ones_col[:], in_=ones_col_f[:])
            ones_col_b = wp.tile([128, 1], bf16)
            nc.vector.memset(ones_col_b[:], 1.0)
            ones_row_f = wp.tile([1, 128], f32)
            nc.vector.memset(ones_row_f[:], 1.0)
            ones_row = wp.tile([1, 128], f32r)
            nc.vector.tensor_copy(out=ones_row[:], in_=ones_row_f[:])
            ones_row_b = wp.tile([1, 64], bf16)
            nc.vector.memset(ones_row_b[:], 1.0)
            ident = wp.tile([128, 128], f32)
            make_identity(nc, ident)
            ident_b = wp.tile([128, 128], bf16)
            nc.vector.tensor_copy(out=ident_b[:], in_=ident[:])

            x2d = dram.tile([BPC, 128, 3, N], f32)

            # head h lives in partition half h%2; slot h//2 (q) / 3+h//2 (k)
            def hslice(qk_tile, h, is_k, t0, tn):
                po = (h % 2) * 64
                slot = (3 if is_k else 0) + h // 2
                return qk_tile[po:po + 64, slot, t0:t0 + tn]

            # ================= STAGE 1 =================
            for s in range(BPC):
                x0T = sb1.tile([128, 3, N], f32, tag="x0T")
                nc.sync.dma_start(out=x0T[:],
                                  in_=dd["xinT"][s, :, :, :].rearrange(
                                      "a p x -> p a x"))
                ln1T = sb1.tile([128, 3, N], f32r, tag="ln1T")
                _ln_fm(nc, sb1, sb3, ppT, x0T, N, ones_col_f, ones_row,
                       cols["gn1"], cols["bn1c"], ln1T, "ln1")
                qkT = sb1.tile([128, 6, N], bf16, tag="qkT")
                for h in range(H):
                    for is_k in (0, 1):
                        po = (h % 2) * 64
                        f0 = (is_k * C) + h * 64
                        for tq in range(2):
                            t0 = tq * TQ
                            ps = ppT.tile([64, TQ], f32, tag="t", name="qkp")
                            for ch in range(3):
                                nc.tensor.matmul(
                                    out=ps[:],
                                    lhsT=wqkv[:, ch, f0:f0 + 64],
                                    rhs=ln1T[:, ch, t0:t0 + TQ],
                                    start=(ch == 0), stop=(ch == 2))
                            if po == 0:
                                nc.vector.tensor_copy(
                                    out=hslice(qkT, h, is_k, t0, TQ),
                                    in_=ps[:])
                            else:
                                qtmp = sb3.tile([64, TQ], bf16, tag="qtmp",
                                                name="qtmp", bufs=2)
                                nc.vector.tensor_copy(out=qtmp[:], in_=ps[:])
                                nc.sync.dma_start(
                                    out=hslice(qkT, h, is_k, t0, TQ),
                                    in_=qtmp[:])
                v1 = sb1.tile([128, 5, H, HD], bf16, tag="v1")
                for j, tk in enumerate(TKS):
                    t0 = j * 128
                    ps = ppT.tile([128, C], f32, tag="t", name="vp")
                    for ch in range(3):
                        nc.tensor.matmul(
                            out=ps[:tk, :], lhsT=ln1T[:, ch, t0:t0 + tk],
                            rhs=wqkv[:, ch, 2 * C:3 * C],
                            start=(ch == 0), stop=(ch == 2))
                    nc.vector.tensor_copy(
                        out=v1[:tk, j, :, :],
                        in_=ps[:tk, :].rearrange("p (h d) -> p h d", h=H))
                mt = sb1.tile([128, 5, N], bf16, tag="maskt")
                for j, tk in enumerate(TKS):
                    nc.sync.dma_start(out=mt[:tk, j, :],
                                      in_=dd["maskT"][s, j * 128:j * 128 + tk, :])
                attn_nT = sb1.tile([128, 3, N], f32r, tag="ln1T",
                                   name="attn_nT")
                for h in range(H):
                    po = (h % 2) * 64
                    den = [ppP.tile([1, TQ], f32, tag="p", name=f"den{t}")
                           for t in range(2)]
                    att = [ppP.tile([64, TQ], f32, tag="p", name=f"att{t}")
                           for t in range(2)]
                    for j, tk in enumerate(TKS):
                        t0 = j * 128
                        for tq in range(2):
                            q0 = tq * TQ
                            st = ppT.tile([128, TQ], f32, tag="t", name="st")
                            nc.tensor.matmul(out=st[:tk, :],
                                             lhsT=hslice(qkT, h, 1, t0, tk),
                                             rhs=hslice(qkT, h, 0, q0, TQ),
                                             start=True, stop=True)
                            lg = sb3.tile([128, TQ], f32, tag="lg", name="lg", bufs=1)
                            nc.vector.scalar_tensor_tensor(
                                out=lg[:tk, :], in0=st[:tk, :], scalar=SCALE,
                                in1=mt[:tk, j, q0:q0 + TQ],
                                op0=OP.mult, op1=OP.add)
                            ex = sb3.tile([128, TQ], bf16, tag="ex", name="ex", bufs=2)
                            nc.scalar.activation(out=ex[:tk, :], in_=lg[:tk, :],
                                                 func=ACTF.Exp)
                            nc.tensor.matmul(out=den[tq][:],
                                             lhsT=ones_col_b[:tk, :],
                                             rhs=ex[:tk, :], start=(j == 0),
                                             stop=(j == 4))
                            nc.tensor.matmul(out=att[tq][:],
                                             lhsT=v1[:tk, j, h, :],
                                             rhs=ex[:tk, :], start=(j == 0),
                                             stop=(j == 4))
                    for tq in range(2):
                        q0 = tq * TQ
                        rr = sb3.tile([1, TQ], f32r, tag="rr")
                        with nc.allow_low_precision(reason="softmax recip"):
                            nc.vector.reciprocal(out=rr[:], in_=den[tq][:])
                        bc = ppT.tile([64, TQ], f32, tag="t", name="bcq")
                        nc.tensor.matmul(out=bc[:],
                                         lhsT=ones_row[:1, :64],
                                         rhs=rr[:1, :], start=True, stop=True)
                        bcs = sb3.tile([64, TQ], f32, tag="bcs", name="bcs",
                                       bufs=1)
                        nc.scalar.copy(out=bcs[:], in_=bc[:])
                        if po == 0:
                            nc.vector.tensor_tensor(
                                out=attn_nT[0:64, h // 2, q0:q0 + TQ],
                                in0=att[tq][:], in1=bcs[:], op=OP.mult)
                        else:
                            ntmp = sb3.tile([64, TQ], f32r, tag="ntmp",
                                            name="ntmp", bufs=2)
                            nc.vector.tensor_tensor(out=ntmp[:], in0=att[tq][:],
                                                    in1=bcs[:], op=OP.mult)
                            nc.sync.dma_start(
                                out=attn_nT[64:128, h // 2, q0:q0 + TQ],
                                in_=ntmp[:])
                x1T = x0T
                for f in range(3):
                    for tq in range(2):
                        q0 = tq * TQ
                        ps = ppT.tile([128, TQ], f32, tag="t", name="pjp")
                        for ch in range(3):
                            nc.tensor.matmul(
                                out=ps[:],
                                lhsT=wproj[:, ch, f * 128:(f + 1) * 128],
                                rhs=attn_nT[:, ch, q0:q0 + TQ],
                                start=(ch == 0), stop=(ch == 2))
                        nc.vector.scalar_tensor_tensor(
                            out=x1T[:, f, q0:q0 + TQ], in0=ps[:],
                            scalar=cols["bproj"][:, f:f + 1],
                            in1=x0T[:, f, q0:q0 + TQ], op0=OP.add, op1=OP.add)
                ln2T = sb1.tile([128, 3, N], bf16, tag="ln2T")
                _ln_fm(nc, sb1, sb3, ppT, x1T, N, ones_col_f, ones_row,
                       cols["gn2"], cols["bn2c"], ln2T, "ln2")
                xfnT = sb1.tile([128, 3, N], f32r, tag="xfnT")
                for tq in range(2):
                    q0 = tq * TQ
                    h1T = sb1.tile([128, 12, TQ], bf16, tag="h1T")
                    for fh in range(12):
                        ps = ppT.tile([128, TQ], f32, tag="t", name="f1p")
                        for ch in range(3):
                            nc.tensor.matmul(
                                out=ps[:],
                                lhsT=wfc1[:, ch, fh * 128:(fh + 1) * 128],
                                rhs=ln2T[:, ch, q0:q0 + TQ],
                                start=(ch == 0), stop=(ch == 2))
                        nc.scalar.activation(out=h1T[:, fh, :], in_=ps[:],
                                             func=ACTF.Gelu,
                                             bias=cols["bfc1"][:, fh:fh + 1],
                                             scale=1.0)
                    for f in range(3):
                        ps = ppT.tile([128, TQ], f32, tag="t", name="f2p")
                        for ch in range(12):
                            nc.tensor.matmul(
                                out=ps[:],
                                lhsT=wfc2[:, ch, f * 128:(f + 1) * 128],
                                rhs=h1T[:, ch, :],
                                start=(ch == 0), stop=(ch == 11))
                        nc.scalar.activation(out=xfnT[:, f, q0:q0 + TQ],
                                             in_=ps[:], func=ACTF.Identity,
                                             bias=cols["bfc2"][:, f:f + 1],
                                             scale=1.0)
                x2T = sb1.tile([128, 3, N], f32, tag="x2T")
                for tq in range(2):
                    q0 = tq * TQ
                    psd = ppT.tile([R, TQ], f32, tag="t", name="adp")
                    for ch in range(3):
                        nc.tensor.matmul(out=psd[:], lhsT=wadd[:, ch, :],
                                         rhs=xfnT[:, ch, q0:q0 + TQ],
                                         start=(ch == 0), stop=(ch == 2))
                    d0 = sb3.tile([R, TQ], f32r, tag="d0", name="d0", bufs=1)
                    nc.scalar.activation(out=d0[:], in_=psd[:], func=ACTF.Gelu,
                                         bias=adb_c[:, :1], scale=1.0)
                    for f in range(3):
                        psu = ppT.tile([128, TQ], f32, tag="t", name="aup")
                        nc.tensor.matmul(out=psu[:],
                                         lhsT=wadu[:, f * 128:(f + 1) * 128],
                                         rhs=d0[:], start=True, stop=True)
                        tt = sb3.tile([128, TQ], f32, tag="adt", name="tt", bufs=1)
                        nc.vector.scalar_tensor_tensor(
                            out=tt[:], in0=psu[:],
                            scalar=cols["adub"][:, f:f + 1],
                            in1=xfnT[:, f, q0:q0 + TQ], op0=OP.add, op1=OP.add)
                        nc.vector.scalar_tensor_tensor(
                            out=x2T[:, f, q0:q0 + TQ], in0=tt[:],
                            scalar=gate_c[:, :1], in1=x1T[:, f, q0:q0 + TQ],
                            op0=OP.mult, op1=OP.add)
                nc.sync.dma_start(out=x2d[s, :, :, :], in_=x2T[:])
                x2b = sb1.tile([128, 3, 640], bf16, tag="x2b")
                nc.vector.memset(x2b[:, :, 512:], 0.0)
                nc.vector.tensor_copy(out=x2b[:, :, :N], in_=x2T[:])
                for j, tk in enumerate(TKS):
                    t0 = j * 128
                    tm = sb3.tile([128, 3, 128], bf16, tag="tm", name="tm",
                                  bufs=3)
                    for ch in range(3):
                        nc.sync.dma_start_transpose(
                            out=tm[:, ch, :], in_=x2b[:, ch, t0:t0 + 128])
                    nc.sync.dma_start(
                        out=in_b[s * N + t0:s * N + t0 + tk, :],
                        in_=tm[:tk, :, :].rearrange("t a p -> t (a p)"))

            if "d_x2" in dbg:
                for s in range(BPC):
                    nc.sync.dma_start(out=dbg["d_x2"][s, :, :, :],
                                      in_=x2d[s, :, :, :].rearrange("p a x -> a p x"))

            # ================= ALLGATHER =================
            if profile_no_collective:
                # TimelineSim can't model collectives; stand in a same-size
                # DRAM->DRAM copy so traffic/deps stay comparable
                nc.sync.dma_start(out=table[:ROWS, :], in_=in_b[:, :])
            else:
                nc.gpsimd.collective_compute(
                    "AllGather", OP.bypass,
                    replica_groups=[list(range(N_CORES))],
                    ins=[in_b.opt()], outs=[table.opt()])

            # ================= STAGE 2+3 =================
            for s in range(BPC):
                cix = sb1.tile([S, 1], i32, tag="cix")
                nc.sync.dma_start(out=cix[:],
                                  in_=dd["cidx"][s * S:(s + 1) * S, :])
                xc = sb1.tile([S, C], bf16, tag="xc")
                nc.gpsimd.indirect_dma_start(
                    out=xc[:], out_offset=None, in_=table[:],
                    in_offset=bass.IndirectOffsetOnAxis(ap=cix[:, :1], axis=0))
                xcT = sb1.tile([128, 3, S], bf16, tag="xcT")
                for ch in range(3):
                    nc.sync.dma_start_transpose(
                        out=xcT[:, ch, :], in_=xc[:, ch * 128:(ch + 1) * 128])
                vis_xT = sb1.tile([128, 3, S], f32, tag="vis_xT")

                for cki in range(8):
                    base = (s * 8 + cki) * 512
                    gT = sb2.tile([128, 3, 512], bf16, tag="gT")
                    ln3T = sb1.tile([128, 3, 512], bf16, tag="ln3T")
                    gsubs = []
                    for sub in range(4):
                        ixt = sb3.tile([128, 1], i32, tag="ixt")
                        nc.sync.dma_start(
                            out=ixt[:],
                            in_=dd["idx"][base + sub * 128:
                                          base + sub * 128 + 128, :])
                        g = sb3.tile([128, C], bf16, tag="gsub", name="gsub",
                                     bufs=4)
                        nc.gpsimd.indirect_dma_start(
                            out=g[:], out_offset=None, in_=table[:],
                            in_offset=bass.IndirectOffsetOnAxis(
                                ap=ixt[:, :1], axis=0))
                        gsubs.append(g)
                        if s == 0 and cki == 0 and sub == 0 and "d_g0" in dbg:
                            og = sb3.tile([128, C], f32, tag="l3", name="dbgg",
                                          bufs=2)
                            nc.vector.tensor_copy(out=og[:], in_=g[:])
                            nc.sync.dma_start(out=dbg["d_g0"][:, :], in_=og[:])
                        for ch in range(3):
                            pt = ppT.tile([128, 128], bf16, tag="t", name="gtp")
                            nc.tensor.transpose(
                                out=pt[:], in_=g[:, ch * 128:(ch + 1) * 128],
                                identity=ident_b[:])
                            nc.vector.tensor_copy(
                                out=gT[:, ch, sub * 128:sub * 128 + 128],
                                in_=pt[:])
                    # LN3 feature-major on gT (stats via PE ones-matmuls)
                    _ln_fm(nc, sb1, sb3, ppT, gT, 512, ones_col_b, ones_row,
                           cols["gn3r"], cols["bn3r"], ln3T, "ln3",
                           ones_sq=ones_col_f)
                    qk2 = sb1.tile([128, 6, 512], bf16, tag="qk2")
                    for h in range(H):
                        for is_k in (0, 1):
                            po = (h % 2) * 64
                            f0 = (is_k * C) + h * 64
                            ps = ppT.tile([64, 512], f32, tag="t", name="qk2p")
                            for ch in range(3):
                                nc.tensor.matmul(
                                    out=ps[:],
                                    lhsT=wa1qkv[:, ch, f0:f0 + 64],
                                    rhs=ln3T[:, ch, :],
                                    start=(ch == 0), stop=(ch == 2))
                            if po == 0:
                                nc.scalar.copy(
                                    out=hslice(qk2, h, is_k, 0, 512), in_=ps[:])
                            else:
                                qtmp = sb3.tile([64, 512], bf16, tag="qtmp2",
                                                name="qtmp2", bufs=1)
                                nc.scalar.copy(out=qtmp[:], in_=ps[:])
                                nc.sync.dma_start(
                                    out=hslice(qk2, h, is_k, 0, 512),
                                    in_=qtmp[:])
                    v2 = sb1.tile([128, 4, H, HD], bf16, tag="v2")
                    for sub in range(4):
                        ps = ppT.tile([128, C], f32, tag="t", name="v2p")
                        for ch in range(3):
                            nc.tensor.matmul(
                                out=ps[:],
                                lhsT=ln3T[:, ch, sub * 128:sub * 128 + 128],
                                rhs=wa1qkv[:, ch, 2 * C:3 * C],
                                start=(ch == 0), stop=(ch == 2))
                        nc.vector.tensor_copy(
                            out=v2[:, sub, :, :],
                            in_=ps[:].rearrange("p (h d) -> p h d", h=H))
                    # block-diagonal attention, batched over the 4 sub-blocks
                    at2 = sb1.tile([128, 3, 512], bf16, tag="at2")
                    for h in range(H):
                        po = (h % 2) * 64
                        stb = ppT.tile([128, 512], f32, tag="t", name="st2")
                        for sub in range(4):
                            o0 = sub * 128
                            nc.tensor.matmul(out=stb[:, o0:o0 + 128],
                                             lhsT=hslice(qk2, h, 1, o0, 128),
                                             rhs=hslice(qk2, h, 0, o0, 128),
                                             start=True, stop=True)
                        lg = sb3.tile([128, 512], f32, tag="lg2", name="lg2",
                                      bufs=1)
                        nc.vector.scalar_tensor_tensor(
                            out=lg[:], in0=stb[:], scalar=SCALE,
                            in1=bmask[:], op0=OP.mult, op1=OP.add)
                        ex = sb3.tile([128, 512], bf16, tag="ex2", name="ex2",
                                      bufs=2)
                        nc.scalar.activation(out=ex[:], in_=lg[:], func=ACTF.Exp)
                        den = ppP.tile([1, 512], f32, tag="p", name="den2")
                        nc.tensor.matmul(out=den[:], lhsT=ones_col_b[:, :1],
                                         rhs=ex[:], start=True, stop=True)
                        att = ppP.tile([64, 512], f32, tag="p", name="att2")
                        for sub in range(4):
                            o0 = sub * 128
                            nc.tensor.matmul(out=att[:, o0:o0 + 128],
                                             lhsT=v2[:, sub, h, :],
                                             rhs=ex[:, o0:o0 + 128],
                                             start=True, stop=True)
                        rr = sb3.tile([1, 512], bf16, tag="rr2", name="rr2",
                                      bufs=1)
                        with nc.allow_low_precision(reason="softmax recip"):
                            nc.vector.reciprocal(out=rr[:], in_=den[:])
                        bcs = sb3.tile([64, 512], bf16, tag="bcs2",
                                       name="bcs2", bufs=1)
                        nc.gpsimd.partition_broadcast(bcs[:], rr[:1, :],
                                                      channels=64)
                        if po == 0:
                            nc.vector.tensor_tensor(
                                out=at2[0:64, h // 2, :],
                                in0=att[:], in1=bcs[:], op=OP.mult)
                        else:
                            ntmp = sb3.tile([64, 512], bf16, tag="ntmp2",
                                            name="ntmp2", bufs=1)
                            nc.vector.tensor_tensor(out=ntmp[:], in0=att[:],
                                                    in1=bcs[:], op=OP.mult)
                            nc.sync.dma_start(out=at2[64:128, h // 2, :],
                                              in_=ntmp[:])
                    if s == 0 and cki == 0 and "d_at2" in dbg:
                        for ch in range(3):
                            ot = sb3.tile([128, 512], f32, tag="xnn",
                                          name="dbg_at", bufs=1)
                            nc.vector.tensor_copy(out=ot[:], in_=at2[:, ch, :])
                            nc.sync.dma_start(out=dbg["d_at2"][ch, :, :], in_=ot[:])
                    lc_all = sb3.tile([128, 3, 16], f32, tag="lcall",
                                      name="lcall", bufs=2)
                    for f in range(3):
                        ps = ppT.tile([128, 512], f32, tag="t", name="pj2")
                        for ch in range(3):
                            nc.tensor.matmul(
                                out=ps[:],
                                lhsT=wa1proj[:, ch, f * 128:(f + 1) * 128],
                                rhs=at2[:, ch, :], start=(ch == 0),
                                stop=(ch == 2))
                        xnn = sb3.tile([128, 512], f32, tag="xnn", name="xnn",
                                       bufs=1)
                        nc.vector.scalar_tensor_tensor(
                            out=xnn[:], in0=ps[:],
                            scalar=cols["ba1proj"][:, f:f + 1],
                            in1=gT[:, f, :], op0=OP.add, op1=OP.add)
                        pm = sb3.tile([128, 16], f32, tag="pm")
                        nc.vector.tensor_reduce(
                            out=pm[:],
                            in_=xnn[:].rearrange("p (g k) -> p g k", k=GK),
                            axis=AX.X, op=OP.max)
                        pa = sb3.tile([128, 16], f32, tag="pa")
                        nc.vector.tensor_reduce(
                            out=pa[:],
                            in_=xnn[:].rearrange("p (g k) -> p g k", k=GK),
                            axis=AX.X, op=OP.add)
                        lc = sb3.tile([128, 16], f32, tag="lc")
                        nc.vector.scalar_tensor_tensor(
                            out=lc[:], in0=pa[:], scalar=1.0 / GK, in1=pm[:],
                            op0=OP.mult, op1=OP.add)
                        nc.scalar.activation(out=lc_all[:, f, :], in_=lc[:],
                                             func=ACTF.Gelu,
                                             bias=cols["bnbias"][:, f:f + 1],
                                             scale=cols["bnscale"][:, f:f + 1])
                    c0 = cki * 16
                    nc.vector.scalar_tensor_tensor(
                        out=vis_xT[:, :, c0:c0 + 16],
                        in0=xcT[:, :, c0:c0 + 16], scalar=0.4, in1=lc_all[:],
                        op0=OP.mult, op1=OP.add)

                # ---- stage 3 ----
                vis_x = sb1.tile([128, 3, S], f32r, tag="vis_x")
                for ch in range(3):
                    pt = ppT.tile([128, 128], f32, tag="t", name="vtp")
                    nc.tensor.transpose(out=pt[:], in_=vis_xT[:, ch, :],
                                        identity=ident[:])
                    nc.vector.tensor_copy(out=vis_x[:, ch, :], in_=pt[:])
                if "d_vis" in dbg:
                    for ch in range(3):
                        ot = sb3.tile([128, S], f32, tag="ob", name="dbv", bufs=2)
                        nc.vector.tensor_copy(out=ot[:], in_=vis_x[:, ch, :])
                        nc.sync.dma_start(out=dbg["d_vis"][s, ch, :, :],
                                          in_=ot[:])
                c2t = sb1.tile([3, S], f32, tag="c2t")
                nc.sync.dma_start(out=c2t[:], in_=dd["c2T"][s, :, :])
                c1t = sb1.tile([3, G], f32, tag="c1t")
                nc.sync.dma_start(out=c1t[:], in_=dd["c1T"][s, :, :])
                n1r = sb1.tile([1, G], f32, tag="n1r")
                nc.sync.dma_start(out=n1r[:], in_=dd["n1sq"][s, :, :])
                n1rr = sb1.tile([1, G], f32r, tag="n1rr")
                nc.vector.tensor_copy(out=n1rr[:], in_=n1r[:])
                n2c = sb1.tile([S, 1], f32, tag="n2c")
                nc.sync.dma_start(out=n2c[:], in_=dd["n2sq"][s, :, :])
                psd = ppP.tile([S, G], f32, tag="p", name="dps")
                nc.tensor.matmul(out=psd[:], lhsT=c2t[:3, :], rhs=c1t[:3, :],
                                 start=True, stop=True)
                nb = _bcast(nc, ppT, ones_row, n1rr, G)
                nbs = sb1.tile([S, G], f32, tag="nbs")
                nc.scalar.copy(out=nbs[:], in_=nb[:])
                dT = sb1.tile([S, G], f32, tag="dT")
                nc.vector.scalar_tensor_tensor(out=dT[:], in0=psd[:],
                                               scalar=-2.0, in1=nbs[:],
                                               op0=OP.mult, op1=OP.add)
                nc.vector.tensor_scalar(out=dT[:], in0=dT[:],
                                        scalar1=n2c[:, :1], scalar2=None,
                                        op0=OP.add)
                rT = sb1.tile([S, G], f32r, tag="rT")
                with nc.allow_low_precision(reason="propagate recip"):
                    nc.vector.reciprocal(out=rT[:], in_=dT[:])
                pss = ppP.tile([1, G], f32, tag="p", name="rsum")
                nc.tensor.matmul(out=pss[:], lhsT=ones_col[:, :1], rhs=rT[:],
                                 start=True, stop=True)
                rs = sb1.tile([1, G], f32r, tag="rs")
                with nc.allow_low_precision(reason="propagate recip"):
                    nc.vector.reciprocal(out=rs[:], in_=pss[:])
                rb = _bcast(nc, ppT, ones_row, rs, G)
                wT = sb1.tile([S, G], f32r, tag="wTn")
                nc.vector.tensor_tensor(out=wT[:], in0=rT[:], in1=rb[:],
                                        op=OP.mult)
                xgT = sb1.tile([128, 3, G], f32, tag="xgT")
                nc.sync.dma_start(out=xgT[:], in_=x2d[s, :, :, T:])
                nxT = sb1.tile([128, 3, G], f32r, tag="nxT")
                for ch in range(3):
                    psi = ppT.tile([128, G], f32, tag="t", name="ips")
                    nc.tensor.matmul(out=psi[:], lhsT=vis_x[:, ch, :],
                                     rhs=wT[:], start=True, stop=True)
                    nc.vector.scalar_tensor_tensor(
                        out=nxT[:, ch, :], in0=psi[:], scalar=0.4,
                        in1=xgT[:, ch, :], op0=OP.mult, op1=OP.add)
                psa = ppT.tile([R, G], f32, tag="t", name="a1dp")
                for ch in range(3):
                    nc.tensor.matmul(out=psa[:], lhsT=wa1d[:, ch, :],
                                     rhs=nxT[:, ch, :], start=(ch == 0),
                                     stop=(ch == 2))
                d1 = sb1.tile([R, G], f32r, tag="d1")
                nc.scalar.activation(out=d1[:], in_=psa[:], func=ACTF.Gelu,
                                     bias=a1db_c[:, :1], scale=1.0)
                for ch in range(3):
                    psu = ppT.tile([128, G], f32, tag="t", name="a1up")
                    nc.tensor.matmul(out=psu[:],
                                     lhsT=wa1u[:, ch * 128:(ch + 1) * 128],
                                     rhs=d1[:], start=True, stop=True)
                    oT = sb3.tile([128, G], f32, tag="oT", name="oT", bufs=2)
                    nc.vector.scalar_tensor_tensor(
                        out=oT[:], in0=psu[:],
                        scalar=cols["a1ub"][:, ch:ch + 1],
                        in1=nxT[:, ch, :], op0=OP.add, op1=OP.add)
                    for j in range(4):
                        pt = ppT.tile([128, 128], f32, tag="t", name="otp")
                        nc.tensor.transpose(out=pt[:],
                                            in_=oT[:, j * 128:(j + 1) * 128],
                                            identity=ident[:])
                        ob = sb3.tile([128, 128], f32, tag="ob", name="ob", bufs=2)
                        nc.vector.tensor_copy(out=ob[:], in_=pt[:])
                        nc.sync.dma_start(
                            out=y[s, j * 128:(j + 1) * 128,
                                  ch * 128:(ch + 1) * 128],
                            in_=ob[:])

    nc.compile()
    return nc


_CACHE = {}


def _get_nc():
    if "nc" not in _CACHE:
        _CACHE["nc"] = build()
    return _CACHE["nc"]


def prep_inputs(inputs):
    """Host-side prep: shard over batch, transpose to device layouts."""
    xx = {k: np.asarray(v) for k, v in inputs.items()}
    bf = ml_dtypes.bfloat16
    shared = {}
    shared["wqkvT"] = np.ascontiguousarray(xx["qkv_w"].T).reshape(3, 128, 3 * C)
    shared["wprojT"] = np.ascontiguousarray(xx["proj_w"].T).reshape(3, 128, C)
    shared["bproj"] = xx["proj_b"].reshape(3, 128)
    shared["wfc1T"] = np.ascontiguousarray(xx["fc1_w"].T).reshape(
        3, 128, H4).astype(bf)
    shared["bfc1"] = xx["fc1_b"].reshape(12, 128)
    shared["wfc2T"] = np.ascontiguousarray(xx["fc2_w"].T).reshape(
        12, 128, C).astype(bf)
    shared["bfc2"] = xx["fc2_b"].reshape(3, 128)
    shared["waddT"] = np.ascontiguousarray(xx["ad_dw"].T).reshape(3, 128, R)
    shared["adb"] = xx["ad_db"].reshape(R, 1)
    shared["waduT"] = np.ascontiguousarray(xx["ad_uw"].T).reshape(R, C)
    shared["adub"] = xx["ad_ub"].reshape(3, 128)
    shared["wa1dT"] = np.ascontiguousarray(xx["ad1_dw"].T).reshape(3, 128, R)
    shared["a1db"] = xx["ad1_db"].reshape(R, 1)
    shared["wa1uT"] = np.ascontiguousarray(xx["ad1_uw"].T).reshape(R, C)
    shared["a1ub"] = xx["ad1_ub"].reshape(3, 128)
    shared["wa1qkvT"] = np.ascontiguousarray(xx["a1_qkv_w"].T).reshape(
        3, 128, 3 * C).astype(bf)
    shared["wa1projT"] = np.ascontiguousarray(xx["a1_proj_w"].T).reshape(
        3, 128, C).astype(bf)
    shared["ba1proj"] = xx["a1_proj_b"].reshape(3, 128)
    shared["gn1"] = xx["n1_g"].reshape(3, 128)
    shared["bn1c"] = xx["n1_b"].reshape(3, 128)
    shared["gn2"] = xx["n2_g"].reshape(3, 128)
    shared["bn2c"] = xx["n2_b"].reshape(3, 128)
    shared["gn3r"] = xx["n3_g"].reshape(3, 128)
    shared["bn3r"] = xx["n3_b"].reshape(3, 128)
    shared["bnscale"] = (xx["bn_g"] / np.sqrt(np.float32(1.0 + 1e-5))
                         ).reshape(3, 128)
    shared["bnbias"] = xx["bn_b"].reshape(3, 128)
    shared["gate"] = xx["ad_gate"].reshape(1, 1)
    bm = np.full((128, 128), NEG, np.float32)
    for g in range(4):
        bm[g * 32:(g + 1) * 32, g * 32:(g + 1) * 32] = 0.0
    shared["blockmask"] = np.tile(bm, (1, 4))
    shared = {k: np.ascontiguousarray(v) for k, v in shared.items()}

    prompt = xx["prompt_embeddings"]
    idx = xx["idx"].reshape(B, S, GK)
    cidx = xx["center_idx"].reshape(B, S)
    maps = []
    for c in range(N_CORES):
        sl = slice(c * BPC, (c + 1) * BPC)
        xin = np.concatenate(
            [np.broadcast_to(prompt[None], (BPC, T, C)), xx["x"][sl]], axis=1)
        m = {}
        m["xinT"] = np.ascontiguousarray(xin.transpose(0, 2, 1)).reshape(
            BPC, 3, 128, N)
        mp = np.zeros((BPC, N, N), np.float32)
        mp[:, :G, :G] = NEG * xx["mask"][sl]
        m["maskT"] = np.ascontiguousarray(mp.transpose(0, 2, 1)).astype(bf)
        m["idx"] = np.ascontiguousarray(idx[sl].reshape(-1, 1))
        m["cidx"] = np.ascontiguousarray(cidx[sl].reshape(-1, 1))
        c1 = xx["center1"][sl]
        c2 = xx["center2"][sl]
        m["c1T"] = np.ascontiguousarray(c1.transpose(0, 2, 1))
        m["n1sq"] = np.ascontiguousarray((c1 ** 2).sum(-1)[:, None, :])
        m["c2T"] = np.ascontiguousarray(c2.transpose(0, 2, 1))
        m["n2sq"] = np.ascontiguousarray((c2 ** 2).sum(-1)[:, :, None] + 1e-8)
        m.update(shared)
        maps.append({k: np.ascontiguousarray(v) for k, v in m.items()})
    return maps


def run(maps, nc=None, debug_outputs=()):
    if nc is None:
        nc = _get_nc()
    res = run_bass_kernel_spmd(nc, maps, core_ids=list(range(N_CORES)))
    return res.results


def kernel(**inputs):
    maps = prep_inputs(inputs)
    results = run(maps)
    out = np.concatenate([r["y"] for r in results], axis=0)
    return out.astype(np.float32)

